# revision 20
# baseline (speedup 1.0000x reference)
"""AlphaRenderer kernel for 8 TRN2 NeuronCores.

Reference computation (per character n of N=4096):
    out[n] = sum_{k in top20 fonts of its text row} softmax_w[k] * alpha_table[font_k, char_class_n] / 255

Rewritten as a dense matmul over all 100 fonts with a top-20-masked
softmax weight matrix:
    out[n] = W[ti[n], :] @ alpha_table[:, c_n, :, :]        (W zero outside top-20)

Sharding strategy: shard by CHAR CLASS. Characters are grouped by their
argmax char class; each class group becomes a [<=KCAP, 100] weight block
that multiplies that class's [100, 4096] alpha-table slice. The 164 MB
table is then read exactly once across the chip (vs 8.2x for the naive
per-character gather). Class chunks are packed into S slots per core.

Device layout choices (all DMA-bandwidth driven):
  - fonts padded 100->128 partitions: the HWDGE splits a DMA into
    contiguous partition blocks, and only P=128 aligns the 16 SDMA
    engines with their SBUF ports.
  - slot PAIRS share PSUM tiles via column-tiled matmuls
    (tile_position=(0,64)), so PSUM->SBUF casts and output DMAs run
    the full 128-partition width.
  - the host interleaves each pair's two table slices row-wise
    ([128, 2*4096] bf16 = 16 KB contiguous per partition) and the
    output groups two pairs the same way: bigger DMA descriptors,
    better per-engine bandwidth.
  - input stream on Sync HWDGE, output stream on GpSimd SWDGE so
    output DMAs never head-of-line-block the table stream.
"""
import os
from contextlib import ExitStack

import ml_dtypes
import numpy as np

import concourse.bass as bass
import concourse.mybir as mybir
import concourse.tile as tile
from concourse import bacc
from concourse.bass_utils import run_bass_kernel_spmd

BF16 = np.dtype(ml_dtypes.bfloat16)

NCORES = 8
F = 100          # fonts
FP = 128         # fonts padded to full partition width
C = 100          # char classes
N = 4096         # characters
HW = 4096        # 64*64 pixels
TOPK = 20
KCAP = 64        # rows per class slot (seed-0 max class count is 56)
NT = 512         # matmul free-dim tile (one PSUM bank of fp32)
PS = 1024        # psum tile free dim: 2 banks, 1 full-width copy

_NC_CACHE: dict = {}
LAST_RESULT = None
USE_RAW = os.environ.get("ALPHA_USE_RAW", "1") == "1"

NRT = 6          # rhs (pair table) buffers
NOG = 4          # output pair buffers


def _build_raw(S: int):
    """Raw-bass pipeline: hand-rolled semaphores, no Tile drain tail.

    Engine roles: Sync issues the table stream (HWDGE), GpSimd the
    output stream (SWDGE), Tensor the matmuls, Vector/Scalar split the
    PSUM->SBUF bf16 cast copies (even/odd psum tiles -> disjoint banks).
    """
    key = ("raw", S)
    if key in _NC_CACHE:
        return _NC_CACHE[key]
    dt = mybir.dt.bfloat16
    npairs = (S + 1) // 2
    ngrp = (npairs + 1) // 2
    ntiles = 4 * npairs          # psum tiles, 4 per pair
    nslots_of = lambda p: min(2, S - 2 * p)

    nc = bass.Bass("TRN2", target_bir_lowering=False, debug=False,
                   num_devices=NCORES)
    table = nc.dram_tensor("table", [npairs, FP, 2 * HW], dt,
                           kind="ExternalInput").ap()
    lhsT = nc.dram_tensor("lhsT", [FP, S * KCAP], dt,
                          kind="ExternalInput").ap()
    out = nc.dram_tensor("out", [npairs, 128, HW], dt,
                         kind="ExternalOutput").ap()

    ctx = ExitStack()
    wt = ctx.enter_context(nc.sbuf_tensor("wt", [FP, S * KCAP], dt))
    rts = [ctx.enter_context(nc.sbuf_tensor(f"rt{i}", [FP, 2 * HW], dt))
           for i in range(NRT)]
    ogs = [ctx.enter_context(nc.sbuf_tensor(f"og{i}", [128, HW], dt))
           for i in range(NOG)]
    pts = [ctx.enter_context(nc.psum_tensor(f"pt{i}", [128, PS], mybir.dt.float32))
           for i in range(4)]
    # One DMA-completion semaphore lane per rotating buffer: increments
    # from concurrently-in-flight DMAs interleave, so cumulative
    # thresholds are only sound per-lane (consecutive DMAs on one lane
    # are strictly ordered by the buffer-reuse dependency chain).
    wt_sem = ctx.enter_context(nc.semaphore("wt_sem"))
    in_sems = [ctx.enter_context(nc.semaphore(f"in_sem{i}"))
               for i in range(NRT)]
    mm_sem = ctx.enter_context(nc.semaphore("mm_sem"))
    cpv = ctx.enter_context(nc.semaphore("cpv"))
    cps = ctx.enter_context(nc.semaphore("cps"))
    out_sems = [ctx.enter_context(nc.semaphore(f"out_sem{i}"))
                for i in range(NOG)]

    # copies: global psum tile t handled by vector (t even) / scalar (t odd)
    def copies_done_upto(t):
        """(vector_count, scalar_count) covering tiles [0, t)."""
        return ((t + 1) // 2, t // 2)

    with nc.Block() as block:

        @block.sync
        def _(sync):
            sync.dma_start(wt[:], lhsT[:]).then_inc(wt_sem, 16)
            for p in range(npairs):
                if p >= NRT:
                    sync.wait_ge(mm_sem, 4 * (p - NRT + 1))
                w = HW * nslots_of(p)
                sync.dma_start(rts[p % NRT][:, :w],
                               table[p, :, :w]).then_inc(in_sems[p % NRT], 16)

        @block.tensor
        def _(tensor):
            tensor.wait_ge(wt_sem, 16)
            for p in range(npairs):
                tensor.wait_ge(in_sems[p % NRT], 16 * (p // NRT + 1))
                ns = nslots_of(p)
                for c in range(4):
                    t = 4 * p + c
                    if t >= 4:
                        tprev = t - 4
                        if tprev % 2 == 0:
                            tensor.wait_ge(cpv, tprev // 2 + 1)
                        else:
                            tensor.wait_ge(cps, tprev // 2 + 1)
                    pt = pts[t % 4]
                    last = None
                    for h in range(ns):
                        s = 2 * p + h
                        for n in range(PS // NT):
                            col = h * HW + c * PS + n * NT
                            last = nc.tensor.matmul(
                                pt.ap()[h * 64:h * 64 + 64,
                                        n * NT:(n + 1) * NT],
                                wt.ap()[:, s * KCAP:(s + 1) * KCAP],
                                rts[p % NRT].ap()[:, col:col + NT],
                                start=True, stop=True,
                                tile_position=(0, 64 * h) if ns == 2
                                else None,
                            )
                    last.then_inc(mm_sem, 1)

        def copy_prog(eng, parity, sem):
            done = 0
            for t in range(parity, ntiles, 2):
                p, c = divmod(t, 4)
                ns = nslots_of(p)
                eng.wait_ge(mm_sem, t + 1)
                if p >= NOG and done < p - NOG + 1:
                    eng.wait_ge(out_sems[p % NOG], 16 * ((p - NOG) // NOG + 1))
                    done = p - NOG + 1
                ocol = c * PS
                og = ogs[p % NOG]
                if parity == 0:
                    eng.tensor_copy(og.ap()[:64 * ns, ocol:ocol + PS],
                                    pts[t % 4].ap()[:64 * ns, :]
                                    ).then_inc(sem, 1)
                else:
                    eng.copy(og.ap()[:64 * ns, ocol:ocol + PS],
                             pts[t % 4].ap()[:64 * ns, :]).then_inc(sem, 1)

        @block.vector
        def _(vector):
            copy_prog(vector, 0, cpv)

        @block.scalar
        def _(scalar):
            copy_prog(scalar, 1, cps)

        @block.gpsimd
        def _(gpsimd):
            for p in range(npairs):
                hi = 4 * (p + 1)
                nv, nsc = copies_done_upto(hi)
                gpsimd.wait_ge(cpv, nv)
                gpsimd.wait_ge(cps, nsc)
                gpsimd.dma_start(out[p, :, :],
                                 ogs[p % NOG].ap()[:, :]
                                 ).then_inc(out_sems[p % NOG], 16)
            # outputs landed (also brings out_sems to final values)
            for i in range(NOG):
                ndma = len([p for p in range(npairs) if p % NOG == i])
                gpsimd.wait_ge(out_sems[i], 16 * ndma)

    # Re-runnability: the NEFF may be executed several times (timing
    # loops, traced warm-ups); all waits use absolute thresholds, so
    # every semaphore must return to 0 between runs.
    nc.sync.drain()
    nc.all_engine_barrier()
    nc.clear_and_free_semaphores([wt_sem, mm_sem, cpv, cps]
                                 + in_sems + out_sems)

    nc._raw_ctx = ctx            # keep sbuf/psum/sem contexts alive
    _NC_CACHE[key] = nc
    return nc


def _build(S: int):
    """Per-core SPMD graph: S class slots of KCAP output rows each."""
    if S in _NC_CACHE:
        return _NC_CACHE[S]
    dt = mybir.dt.bfloat16
    npairs = (S + 1) // 2
    ngrp = (npairs + 1) // 2
    nc = bacc.Bacc("TRN2", target_bir_lowering=False, debug=False,
                   num_devices=NCORES)
    table = nc.dram_tensor("table", [npairs, FP, 2 * HW], dt,
                           kind="ExternalInput").ap()
    lhsT = nc.dram_tensor("lhsT", [FP, S * KCAP], dt,
                          kind="ExternalInput").ap()
    out = nc.dram_tensor("out", [ngrp, 128, 2 * HW], dt,
                         kind="ExternalOutput").ap()

    with tile.TileContext(nc) as tc:
        with tc.tile_pool(name="w", bufs=1) as wpool, \
             tc.tile_pool(name="rhs", bufs=5) as rpool, \
             tc.tile_pool(name="ps", bufs=4, space="PSUM") as ppool, \
             tc.tile_pool(name="og", bufs=3) as opool:
            wt = wpool.tile([FP, S * KCAP], dt)
            nc.sync.dma_start(wt[:], lhsT[:])
            og = None
            for p in range(npairs):
                nslots = min(2, S - 2 * p)
                width = HW * nslots
                if p % 2 == 0:
                    og = opool.tile([128, 2 * HW], dt, tag="og")
                rt = rpool.tile([FP, 2 * HW], dt, tag="rhs")
                nc.sync.dma_start(rt[:, :width], table[p, :, :width])
                ocol = (p % 2) * HW
                for c in range(HW // PS):
                    pt = ppool.tile([64 * nslots, PS], mybir.dt.float32,
                                    tag="ps")
                    for h in range(nslots):
                        s = 2 * p + h
                        for n in range(PS // NT):
                            col = h * HW + c * PS + n * NT
                            nc.tensor.matmul(
                                pt[h * 64:h * 64 + 64, n * NT:(n + 1) * NT],
                                wt[:, s * KCAP:(s + 1) * KCAP],
                                rt[:, col:col + NT],
                                start=True, stop=True,
                                tile_position=(0, 64 * h) if nslots == 2
                                else None,
                            )
                    nc.any.tensor_copy(
                        og[:64 * nslots, ocol + c * PS:ocol + (c + 1) * PS],
                        pt[:])
                if p % 2 == 1 or p == npairs - 1:
                    gwidth = HW * (2 if p % 2 == 1 else 1)
                    nc.gpsimd.dma_start(out[p // 2, :, :gwidth],
                                        og[:, :gwidth])
    nc.compile()
    _NC_CACHE[S] = nc
    return nc


def kernel(font_pred, char_labels, char_rec_vec, text_indexes, alpha_table):
    global LAST_RESULT
    BT = font_pred.shape[0] * font_pred.shape[1]

    # --- host: masked-softmax weight matrix [BT, F] ---
    fp = np.asarray(font_pred, np.float32).reshape(BT, F)
    m = fp.max(axis=1, keepdims=True)
    e = np.exp(fp - m)
    sfm = e / e.sum(axis=1, keepdims=True)
    topk = np.argpartition(-fp, TOPK - 1, axis=1)[:, :TOPK]
    M = np.zeros((BT, F), np.float32)
    rows = np.arange(BT)[:, None]
    M[rows, topk] = sfm[rows, topk]
    M *= np.float32(1.0 / 255.0)

    char_idx = np.asarray(char_rec_vec).argmax(axis=1)
    ti = np.asarray(text_indexes).reshape(-1)
    Wc = M[ti]                                   # [N, F] per-char weights

    # --- host: group chars by class, chunk to <=KCAP, pack into cores ---
    chunks = []                                  # (class, np.array(char_ids))
    order = np.argsort(char_idx, kind="stable")
    sorted_cls = char_idx[order]
    starts = np.searchsorted(sorted_cls, np.arange(C), side="left")
    ends = np.searchsorted(sorted_cls, np.arange(C), side="right")
    for c in range(C):
        ids = order[starts[c]:ends[c]]
        for i in range(0, len(ids), KCAP):
            chunks.append((c, ids[i:i + KCAP]))
    S = max(1, -(-len(chunks) // NCORES))
    npairs = (S + 1) // 2
    per_core = [chunks[i::NCORES] for i in range(NCORES)]

    tbl = np.asarray(alpha_table, np.float32).reshape(F, C, HW)
    tbl_bf = tbl.astype(BF16)

    in_maps = []
    slot_ids = []                                # per core, per slot char ids
    for core in range(NCORES):
        table_i = np.zeros((npairs, FP, 2, HW), BF16)
        lhsT_i = np.zeros((FP, S * KCAP), np.float32)
        ids_i = []
        for s, (c, ids) in enumerate(per_core[core]):
            table_i[s // 2, :F, s % 2] = tbl_bf[:, c, :]
            lhsT_i[:F, s * KCAP:s * KCAP + len(ids)] = Wc[ids].T
            ids_i.append(ids)
        in_maps.append({"table": table_i.reshape(npairs, FP, 2 * HW),
                        "lhsT": lhsT_i.astype(BF16)})
        slot_ids.append(ids_i)

    nc = _build_raw(S) if USE_RAW else _build(S)
    res = run_bass_kernel_spmd(nc, in_maps, core_ids=list(range(NCORES)))
    LAST_RESULT = res

    out_full = np.zeros((N, HW), np.float32)
    for core in range(NCORES):
        o = np.asarray(res.results[core]["out"], np.float32)
        for s, ids in enumerate(slot_ids[core]):
            p, h = divmod(s, 2)
            out_full[ids] = o[p, h * 64:h * 64 + len(ids), :]
    return out_full.reshape(N, 1, 1, 64, 64)
